# revision 32
# baseline (speedup 1.0000x reference)
"""Trainium2 Bass kernel for LorentzMultiheadAttention (B=2, N=2048, H=8, D=64, E=512).

Sharding: 8 cores = 2 batches x 4 head-pairs. Core c handles batch b=c//4 and
heads {2*(c%4), 2*(c%4)+1}: projections for its 2 heads, full attention, and
per-head centroid + head-sum. A ReduceScatter over each batch's 4-core group
sums across all 8 heads, leaving each core the final centroid for a slice of
queries. The query dim is processed in 2 chunks of 1024 so chunk 0's collective
overlaps chunk 1's compute.

Performance structure (vs naive):
- All big matmuls run in fp8e4m3 with DoubleRow perf mode (2 contraction rows
  per PE cycle): projections contract E=512 as 2 DR-pairs of 128x2; scores
  contract the 64 q/k dims as [32 partitions x 2]; PV contracts 256 keys per
  call as [128 x 2]. exp() writes fp8 directly from the Activation engine.
- Software pipelining: the PE executes in order, so PV matmuls (which depend
  on exp output) are emitted one key-pair late; the PE streams scores for
  later blocks while the Activation engine exponentiates earlier ones. The
  steady state is Activation-bound (one 1024-wide exp per 128-key block).
- The Lorentz centroid is scale-invariant, so softmax denominators and the
  head-mean divide cancel; no max-subtraction needed (|att| <= ~2).
- Lorentz sign folded into negated K weights; exp(scale*S' + bias) fuses the
  rest into the activation.
"""

import os
import sys

for _p in ("/opt/trn_rl_repo", "/root/.axon_site/_ro/trn_rl_repo"):
    if os.path.isdir(_p) and _p not in sys.path:
        sys.path.insert(0, _p)

import numpy as np

import concourse.bacc as bacc
import concourse.bass as bass
import concourse.mybir as mybir
import concourse.tile as tile

B = 2
N = 2048
H = 8
D = 64
E = 512
DM1 = D - 1  # 63
P = 128
N_CORES = 8
HPC = 2      # heads per core
NCH = 4      # query chunks
CHQ = N // NCH       # 512 queries per chunk
# staggered collectives: AG0 covers chunks 0-1, AG1 chunk 2 (hidden under
# chunk 3's compute), AG2 chunk 3 (small exposed tail)
HALF_CHUNKS = ((0, 1), (2,), (3,))
HALF_Q = (2 * CHQ, CHQ, CHQ)
HALF_OFF = (0, 2 * CHQ, 3 * CHQ)
NHALF = 3

F32 = mybir.dt.float32
BF16 = mybir.dt.bfloat16
FP8 = mybir.dt.float8e4
EXP = mybir.ActivationFunctionType.Exp
SQRT = mybir.ActivationFunctionType.Sqrt
ADD = mybir.AluOpType.add
MULT = mybir.AluOpType.mult
DR = mybir.MatmulPerfMode.DoubleRow
AXX = mybir.AxisListType.X

REPLICA_GROUPS = [[0, 1, 2, 3, 4, 5, 6, 7]]


def _emit(tc, nc, io, scale_val, bias_val):
    from contextlib import ExitStack

    ctx = ExitStack()
    with ctx:
        consts = ctx.enter_context(tc.tile_pool(name="consts", bufs=1))
        sb = ctx.enter_context(tc.tile_pool(name="sb", bufs=1))

        # ---- constants / weights ----
        ident = consts.tile([P, P], BF16)
        nc.sync.dma_start(ident[:], io["ident"].ap())
        mask_lift = consts.tile([P, 2], BF16)
        nc.sync.dma_start(mask_lift[:], io["mask_lift"].ap())

        w_sb = {}
        b_sb = {}
        for nm in ("wq", "wk", "wv"):
            w = consts.tile([P, 4, P], FP8, name=f"{nm}_sb")
            nc.sync.dma_start(w[:], io[nm].ap().rearrange("(c p) m -> p c m", p=P))
            w_sb[nm] = w
        for nm in ("bq", "bk", "bv"):
            bt = consts.tile([P, 1], F32, name=f"{nm}_sb")
            nc.sync.dma_start(bt[:], io[nm].ap().rearrange("(p one) -> p one", one=1))
            b_sb[nm] = bt

        act_scale = -2.0 / scale_val
        act_bias = 2.0 / scale_val + bias_val
        ebias = consts.tile([P, 1], F32)
        nc.vector.memset(ebias[:], act_bias)

        xq8 = sb.tile([P, 4, N], FP8)
        xs8 = sb.tile([P, 4, N], FP8)
        for half in range(2):
            rs = slice(half * 256, (half + 1) * 256)
            cs = slice(2 * half, 2 * half + 2)
            nc.sync.dma_start(
                xq8[:, cs, :],
                io["xq8"].ap()[rs, :].rearrange("(c p) n -> p c n", p=P),
            )
        for half in range(2):
            rs = slice(half * 256, (half + 1) * 256)
            cs = slice(2 * half, 2 * half + 2)
            nc.sync.dma_start(
                xs8[:, cs, :],
                io["xs8"].ap()[rs, :].rearrange("(c p) n -> p c n", p=P),
            )

        # ---- persistent attention operands ----
        # q_dr/k_dr: [32, (h,s), N] fp8, DoubleRow layout: partition p, group
        # 2h+s holds head-h projection dim s*32+p (dim 0 = time).
        q_dr = sb.tile([32, 4, N], FP8)
        k_dr = sb.tile([32, 4, N], FP8)
        # v8: [key-part, mc, 2h*64d] natural-layout values
        v8 = sb.tile([P, 16, P], FP8)

        # ---- Phase A: projections + lifts + DR remap ----
        ctxA = ExitStack()
        psA = ctxA.enter_context(tc.tile_pool(name="psA", bufs=1, space="PSUM"))
        psN = ctxA.enter_context(tc.tile_pool(name="psN", bufs=2, space="PSUM"))
        psT = ctxA.enter_context(tc.tile_pool(name="psT", bufs=2, space="PSUM"))
        sbA = ctxA.enter_context(tc.tile_pool(name="sbA", bufs=1))

        def project(x8, w, bias, xT):
            """proj + bias -> xT [128, N] bf16 (transposed layout), pipelined
            in 1024-col halves so the DVE bias-add overlaps the next matmuls."""
            for hh in range(2):
                ps = psA.tile([P, N // 2], F32, tag="proj", bufs=2)
                for qc in range(2):
                    c0 = hh * 1024 + qc * 512
                    for j in range(2):
                        nc.tensor.matmul(
                            ps[:, qc * 512 : (qc + 1) * 512],
                            lhsT=w[:, 2 * j : 2 * j + 2, :],
                            rhs=x8[:, 2 * j : 2 * j + 2, c0 : c0 + 512],
                            start=(j == 0),
                            stop=(j == 1),
                            perf_mode=DR,
                        )
                nc.vector.tensor_tensor(
                    xT[:, hh * 1024 : (hh + 1) * 1024],
                    ps[:],
                    bias[:].to_broadcast((P, N // 2)),
                    ADD,
                )
            return xT

        def lift_remap(xT, dst_dr):
            """Write fp8 DR-layout projections + time rows into dst_dr."""
            sq = sbA.tile([P, N], BF16, tag="sq")
            nc.vector.tensor_tensor(sq[:], xT[:], xT[:], MULT)
            t8 = sbA.tile([2, N], FP8, tag="t8")
            for s in range(4):
                nrm = psN.tile([2, 512], F32, tag="nrm")
                nc.tensor.matmul(
                    nrm[:],
                    lhsT=mask_lift[:],
                    rhs=sq[:, s * 512 : (s + 1) * 512],
                    start=True,
                    stop=True,
                )
                nc.scalar.activation(
                    t8[:, s * 512 : (s + 1) * 512], nrm[:], SQRT, bias=1.0, scale=1.0
                )
            x8n = sbA.tile([P, N], FP8, tag="x8n")
            nc.vector.tensor_copy(out=x8n[:], in_=xT[:])
            for g in range(4):  # g = 2h + s
                nc.sync.dma_start(
                    dst_dr[:, g, :], x8n[g * 32 : (g + 1) * 32, :]
                )
            for h in range(2):  # overwrite time slot (dim 0 of each head)
                nc.sync.dma_start(dst_dr[0:1, 2 * h, :], t8[h : h + 1, :])

        qT = sbA.tile([P, N], BF16, tag="xT", name="qT", bufs=2)
        project(xq8, w_sb["wq"], b_sb["bq"], qT)
        lift_remap(qT, q_dr)
        kT = sbA.tile([P, N], BF16, tag="xT", name="kT", bufs=2)
        project(xs8, w_sb["wk"], b_sb["bk"], kT)
        lift_remap(kT, k_dr)

        # V: transpose to natural layout, convert fp8, lift per row
        vT = sbA.tile([P, N], BF16, tag="xT", name="vT", bufs=2)
        project(xs8, w_sb["wv"], b_sb["bv"], vT)
        for half in range(2):
            vt_ps = psT.tile([P, 8, P], BF16, tag="vt")
            for j in range(8):
                mt = half * 8 + j
                nc.tensor.matmul(
                    vt_ps[:, j, :],
                    lhsT=vT[:, mt * P : (mt + 1) * P],
                    rhs=ident[:],
                    is_transpose=True,
                    skip_group_check=True,
                )
            nc.vector.tensor_copy(
                out=v8[:, half * 8 : (half + 1) * 8, :], in_=vt_ps[:]
            )
        vsq = sbA.tile([P, 16, P], F32, tag="vsq")
        nc.vector.tensor_tensor(vsq[:], v8[:], v8[:], MULT)
        vn = sb.tile([P, 16, 2, 1], F32)
        nc.vector.tensor_reduce(
            vn[:, :, :, 0],
            vsq[:].rearrange("p t (h d) -> p t h d", h=2),
            axis=AXX,
            op=ADD,
        )
        nc.scalar.activation(
            v8[:].rearrange("p t (h d) -> p t h d", h=2)[:, :, :, 0:1],
            vn[:],
            SQRT,
            bias=1.0,
            scale=1.0,
        )

        ctxA.close()

        # ---- Phase B: attention, 4 query chunks, 2 AllGather halves ----
        psS = ctx.enter_context(tc.tile_pool(name="psS", bufs=3, space="PSUM"))
        psPV = ctx.enter_context(tc.tile_pool(name="psPV", bufs=1, space="PSUM"))
        pP = ctx.enter_context(tc.tile_pool(name="pP", bufs=3))
        sbC = ctx.enter_context(tc.tile_pool(name="sbC", bufs=2))

        dram = ctx.enter_context(tc.tile_pool(name="dram", bufs=1, space="DRAM"))
        cc_in = [
            dram.tile([HALF_Q[c], D], BF16, name=f"ccin{c}") for c in range(NHALF)
        ]
        cc_out = [
            dram.tile(
                [N_CORES, HALF_Q[c], D], BF16, name=f"ccout{c}", addr_space="Shared"
            )
            for c in range(NHALF)
        ]
        fins = []

        for qc in range(NCH):
            # DoubleRow requires dst partition base 0: each head accumulates
            # into its own partition-0-based psum tile.
            pv_a = psPV.tile([64, CHQ], F32, tag="pva")
            pv_b = psPV.tile([64, CHQ], F32, tag="pvb")
            pv_t = (pv_a, pv_b)
            p8t = {}

            def emit_pv(p, h):
                nc.tensor.matmul(
                    pv_t[h][:],
                    lhsT=v8[:, 2 * p : 2 * p + 2, h * 64 : (h + 1) * 64],
                    rhs=p8t[(p, h)][:],
                    start=(p == 0),
                    stop=(p == 7),
                    perf_mode=DR,
                    skip_group_check=True,
                )

            for j in range(8):  # key-block pairs (256 keys each)
                for h in range(HPC):
                    s_ps = psS.tile([P, 2, CHQ], F32, tag="s")
                    for sub in range(2):
                        mc = 2 * j + sub
                        nc.tensor.matmul(
                            s_ps[:, sub, :],
                            lhsT=k_dr[:, 2 * h : 2 * h + 2, mc * P : (mc + 1) * P],
                            rhs=q_dr[:, 2 * h : 2 * h + 2, qc * CHQ : (qc + 1) * CHQ],
                            start=True,
                            stop=True,
                            perf_mode=DR,
                            skip_group_check=True,
                        )
                    p8t[(j, h)] = pP.tile([P, 2, CHQ], FP8, tag=f"p{h}", name=f"p8_{h}")
                    nc.scalar.activation(
                        p8t[(j, h)][:],
                        s_ps[:],
                        EXP,
                        scale=act_scale,
                        bias=ebias[:],
                    )
                # software pipelining: PV one key-pair behind the scores
                if j >= 1:
                    for h in range(HPC):
                        emit_pv(j - 1, h)
            for h in range(HPC):
                emit_pv(7, h)

            # ---- per-head centroid + head-sum (chunk tail) ----
            pv_sba = sbC.tile([64, CHQ], BF16, tag="pvsba")
            nc.vector.tensor_copy(out=pv_sba[:], in_=pv_a[:])
            pv_sbb = sbC.tile([64, CHQ], BF16, tag="pvsbb")
            nc.vector.tensor_copy(out=pv_sbb[:], in_=pv_b[:])
            # transpose scratch: reuse a scores-psum buffer, bitcast to bf16
            trf = psS.tile([P, 2, CHQ], F32, tag="s", name=f"trf{qc}")
            trv = trf.bitcast(BF16)[:, 0, 0 : 4 * P].rearrange(
                "p (t c) -> p t c", c=P
            )
            NT = CHQ // P  # 4 query tiles per chunk
            for j in range(NT):
                for h, src in ((0, pv_sba), (1, pv_sbb)):
                    nc.tensor.matmul(
                        trv[:, j, h * 64 : (h + 1) * 64],
                        lhsT=src[:, j * P : (j + 1) * P],
                        rhs=ident[0:64, 0:64],
                        is_transpose=True,
                        skip_group_check=True,
                    )
            o_nat = sbC.tile([P, NT, P], BF16, tag="onat")
            nc.vector.tensor_copy(out=o_nat[:], in_=trv[:])
            o4 = o_nat[:].rearrange("p t (h d) -> p t h d", h=2)
            sqn = sbC.tile([P, NT, P], BF16, tag="sqn")
            nc.vector.tensor_tensor(sqn[:], o_nat[:], o_nat[:], MULT)
            ssum = sbC.tile([P, NT, 2, 1], F32, tag="ssum")
            nc.vector.tensor_reduce(
                ssum[:, :, :, 0],
                sqn[:].rearrange("p t (h d) -> p t h d", h=2),
                axis=AXX,
                op=ADD,
            )
            t2 = sbC.tile([P, NT, 2, 1], F32, tag="t2")
            nc.vector.tensor_tensor(t2[:], o4[:, :, :, 0:1], o4[:, :, :, 0:1], MULT)
            # inner = ssum - 2*t^2  (negative); den = sqrt(-inner)
            nc.vector.scalar_tensor_tensor(
                ssum[:], t2[:], -2.0, ssum[:], op0=MULT, op1=ADD
            )
            den = sbC.tile([P, NT, 2, 1], F32, tag="den")
            nc.scalar.activation(den[:], ssum[:], SQRT, bias=0.0, scale=-1.0)
            rec = sbC.tile([P, NT, 2, 1], F32, tag="rec")
            nc.vector.reciprocal(rec[:], den[:])
            ov = sbC.tile([P, NT, 2, D], BF16, tag="ov")
            nc.vector.tensor_tensor(
                ov[:], o4[:], rec[:].to_broadcast((P, NT, 2, D)), MULT
            )
            ohs = sbC.tile([P, NT, D], BF16, tag="ohs")
            nc.vector.tensor_tensor(ov[:, :, 0, :], ov[:, :, 0, :], ov[:, :, 1, :], ADD)
            nc.vector.tensor_copy(out=ohs[:], in_=ov[:, :, 0, :])

            half = next(i for i, cs_ in enumerate(HALF_CHUNKS) if qc in cs_)
            off = qc * CHQ - HALF_OFF[half]
            nc.sync.dma_start(
                cc_in[half][off : off + CHQ, :].rearrange("(t p) d -> p t d", p=P),
                ohs[:],
            )
            if qc == HALF_CHUNKS[half][-1]:
                nc.gpsimd.collective_compute(
                    "AllGather",
                    mybir.AluOpType.bypass,
                    replica_groups=REPLICA_GROUPS,
                    ins=[cc_in[half][:].opt()],
                    outs=[cc_out[half][:].opt()],
                )
                ntf = HALF_Q[half] // P
                fin = sbC.tile(
                    [P, 8, ntf, D], BF16, tag=f"fin{half}", name=f"fin{half}"
                )
                # split the gather-in across two queues (descriptor-heavy)
                nc.sync.dma_start(
                    fin[:, 0:4],
                    cc_out[half][0:4].rearrange("r (t p) d -> p r t d", p=P),
                )
                nc.gpsimd.dma_start(
                    fin[:, 4:8],
                    cc_out[half][4:8].rearrange("r (t p) d -> p r t d", p=P),
                )
                fins.append(fin)

        # ---- local group-sum + final centroid per half ----
        for hf in range(NHALF):
            fin = fins[hf]
            ntf = HALF_Q[hf] // P
            for g in range(2):
                fs0 = sbC.tile([P, ntf, D], BF16, tag=f"fs0_{hf}", name="fs0")
                nc.vector.tensor_tensor(
                    fs0[:], fin[:, 4 * g + 0, :, :], fin[:, 4 * g + 1, :, :], ADD
                )
                fs1 = sbC.tile([P, ntf, D], BF16, tag=f"fs1_{hf}", name="fs1")
                nc.vector.tensor_tensor(
                    fs1[:], fin[:, 4 * g + 2, :, :], fin[:, 4 * g + 3, :, :], ADD
                )
                nc.vector.tensor_tensor(fs0[:], fs0[:], fs1[:], ADD)
                fsq = sbC.tile([P, ntf, D], BF16, tag=f"fsq_{hf}", name="fsq")
                nc.vector.tensor_tensor(fsq[:], fs0[:], fs0[:], MULT)
                fss = sbC.tile([P, ntf, 1], F32, tag=f"fss_{hf}", name="fss")
                nc.vector.tensor_reduce(fss[:, :, 0], fsq[:], axis=AXX, op=ADD)
                ft2 = sbC.tile([P, ntf, 1], F32, tag=f"ft2_{hf}", name="ft2")
                nc.vector.tensor_tensor(
                    ft2[:], fs0[:, :, 0:1], fs0[:, :, 0:1], MULT
                )
                nc.vector.scalar_tensor_tensor(
                    fss[:], ft2[:], -2.0, fss[:], op0=MULT, op1=ADD
                )
                fden = sbC.tile([P, ntf, 1], F32, tag=f"fden_{hf}", name="fden")
                nc.scalar.activation(fden[:], fss[:], SQRT, bias=0.0, scale=-1.0)
                frec = sbC.tile([P, ntf, 1], F32, tag=f"frec_{hf}", name="frec")
                nc.vector.reciprocal(frec[:], fden[:])
                fout = sbC.tile([P, ntf, D], F32, tag=f"fout_{hf}", name="fout")
                nc.vector.tensor_tensor(
                    fout[:], fs0[:], frec[:].to_broadcast((P, ntf, D)), MULT
                )
                nc.sync.dma_start(
                    io["out"]
                    .ap()[g, HALF_OFF[hf] : HALF_OFF[hf] + HALF_Q[hf], :]
                    .rearrange("(t p) d -> p t d", p=P),
                    fout[:],
                )


def _build(scale_val, bias_val):
    nc = bacc.Bacc(num_devices=N_CORES)
    io = {}
    io["xq8"] = nc.declare_dram_parameter("xq8", [E, N], FP8, isOutput=False)
    io["xs8"] = nc.declare_dram_parameter("xs8", [E, N], FP8, isOutput=False)
    for nm in ("wq", "wk", "wv"):
        io[nm] = nc.declare_dram_parameter(nm, [E, P], FP8, isOutput=False)
    for nm in ("bq", "bk", "bv"):
        io[nm] = nc.declare_dram_parameter(nm, [P], F32, isOutput=False)
    io["ident"] = nc.declare_dram_parameter("ident", [P, P], BF16, isOutput=False)
    io["mask_lift"] = nc.declare_dram_parameter("mask_lift", [P, 2], BF16, isOutput=False)
    io["out"] = nc.declare_dram_parameter("out", [B, N, D], F32, isOutput=True)

    with tile.TileContext(nc) as tc:
        _emit(tc, nc, io, scale_val, bias_val)
    nc.compile()
    return nc


_BUILD_CACHE = {}


def _get_nc(scale_val, bias_val):
    key = (float(scale_val), float(bias_val))
    if key not in _BUILD_CACHE:
        _BUILD_CACHE[key] = _build(*key)
    return _BUILD_CACHE[key]


def _pad_wT(w_heads):
    """[126, 512] spatial weights for 2 heads -> [512, 128] transposed with
    zero cols at 0/64 (time slots)."""
    out = np.zeros((E, P), dtype=np.float32)
    out[:, 1:64] = w_heads[0:DM1, :].T
    out[:, 65:128] = w_heads[DM1 : 2 * DM1, :].T
    return np.ascontiguousarray(out)


def _pad_b(b_heads):
    out = np.zeros((P,), dtype=np.float32)
    out[1:64] = b_heads[0:DM1]
    out[65:128] = b_heads[DM1 : 2 * DM1]
    return out


def make_in_maps(
    query_input, source_input, Wq_w, Wq_b, Wk_w, Wk_b, Wv_w, Wv_b, scale, bias
):
    import ml_dtypes

    F8 = ml_dtypes.float8_e4m3
    BF = ml_dtypes.bfloat16
    ident = np.eye(P, dtype=BF)
    mask_lift = np.zeros((P, 2), dtype=BF)
    mask_lift[1:64, 0] = 1.0
    mask_lift[65:128, 1] = 1.0

    in_maps = []
    for c in range(N_CORES):
        b = c // 4
        h0 = 2 * (c % 4)
        sl = slice(h0 * DM1, (h0 + 2) * DM1)
        m = {
            "xq8": np.ascontiguousarray(query_input[b].T).astype(F8),
            "xs8": np.ascontiguousarray(source_input[b].T).astype(F8),
            "wq": _pad_wT(Wq_w[sl]).astype(F8),
            "wk": _pad_wT(-Wk_w[sl]).astype(F8),  # Lorentz sign folded into K
            "wv": _pad_wT(Wv_w[sl]).astype(F8),
            "bq": _pad_b(Wq_b[sl]),
            "bk": _pad_b(-Wk_b[sl]),
            "bv": _pad_b(Wv_b[sl]),
            "ident": ident,
            "mask_lift": mask_lift,
        }
        in_maps.append(m)
    return in_maps


def kernel(
    query_input,
    source_input,
    Wq_w,
    Wq_b,
    Wk_w,
    Wk_b,
    Wv_w,
    Wv_b,
    scale,
    bias,
    _trace=False,
):
    scale_val = float(np.asarray(scale).reshape(-1)[0])
    bias_val = float(np.asarray(bias).reshape(-1)[0]) if np.asarray(bias).size else 0.0

    nc = _get_nc(scale_val, bias_val)
    in_maps = make_in_maps(
        query_input, source_input, Wq_w, Wq_b, Wk_w, Wk_b, Wv_w, Wv_b, scale, bias
    )

    from concourse.bass_utils import run_bass_kernel_spmd

    res = run_bass_kernel_spmd(
        nc, in_maps, core_ids=list(range(N_CORES)), trace=_trace
    )

    # every core computes the full output (8-core AllGather + local sums);
    # read core 0's copy
    out = np.asarray(res.results[0]["out"]).astype(np.float32)
    if _trace:
        kernel.last_exec_time_ns = res.exec_time_ns
        kernel.last_results = res
    return out
